# revision 5
# baseline (speedup 1.0000x reference)
"""Causal self-attention Trainium2 kernel.

Problem: B=4, T=2048, D=2048, H=16 heads x 128 head-size, fp32.
Sharding: 8 cores = 4 batches x 2 head-groups (8 heads each).
Each core computes, for its (batch b, head-group g):
  qkv projections -> causal attention (flash-style, transposed layout)
  -> partial output projection  out_partial = O_g @ w_o[g]
Host sums the two group partials per batch and adds bias terms.

All matmuls run as float32r (tf32-like) on the PE array.
"""

import sys

sys.path.insert(0, "/opt/trn_rl_repo")

import numpy as np

import concourse.bass as bass
import concourse.bacc as bacc
import concourse.mybir as mybir
from concourse.tile import TileContext
from concourse.bass_utils import run_bass_kernel_spmd

DT = mybir.dt
AF = mybir.ActivationFunctionType

B, T, D = 4, 2048, 2048
H_PER_CORE = 8          # heads per core
DH = 128                # head size
HD = H_PER_CORE * DH    # 1024 hidden per core
KT = D // 128           # 16 contraction tiles
TQ = T // 512           # 4 query chunks of 512
TT = T // 128           # 16 t tiles
SCALE = 1.0 / np.sqrt(DH)
NEG = -1e10


def build_nc():
    nc = bacc.Bacc("TRN2", target_bir_lowering=False, debug=False)
    f32 = DT.float32
    f32r = DT.float32r

    xT = nc.dram_tensor("xT", [D, T], f32r, kind="ExternalInput")
    wq = nc.dram_tensor("wq", [D, HD], f32r, kind="ExternalInput")
    wk = nc.dram_tensor("wk", [D, HD], f32r, kind="ExternalInput")
    wv = nc.dram_tensor("wv", [D, HD], f32r, kind="ExternalInput")
    wo = nc.dram_tensor("wo", [HD, D], f32r, kind="ExternalInput")
    bq = nc.dram_tensor("bq", [128, H_PER_CORE], f32, kind="ExternalInput")
    bk = nc.dram_tensor("bk", [128, H_PER_CORE], f32, kind="ExternalInput")
    maskadd = nc.dram_tensor("maskadd", [128, 1024], f32, kind="ExternalInput")
    ones_col = nc.dram_tensor("ones_col", [128, 1], f32r, kind="ExternalInput")
    ones_row = nc.dram_tensor("ones_row", [1, 128], f32r, kind="ExternalInput")
    outp = nc.dram_tensor("out", [T, D], f32, kind="ExternalOutput")

    qTs = nc.dram_tensor("qTs", [H_PER_CORE, 128, T], f32r, kind="Internal")
    kTs = nc.dram_tensor("kTs", [H_PER_CORE, 128, T], f32r, kind="Internal")
    vs = nc.dram_tensor("vs", [T, HD], f32r, kind="Internal")
    oTs = nc.dram_tensor("oTs", [H_PER_CORE, 128, T], f32r, kind="Internal")

    with TileContext(nc) as tc:
        # ---------------- Phase A1: qT, kT = (x @ wq/wk + b)^T ----------
        with (
            tc.tile_pool(name="xt_pool", bufs=1) as xt_pool,
            tc.tile_pool(name="wcol_pool", bufs=2) as wcol_pool,
            tc.tile_pool(name="bias_pool", bufs=1) as bias_pool,
            tc.tile_pool(name="qk_stage", bufs=3) as qk_stage,
            tc.tile_pool(name="ps_a", bufs=4, space="PSUM") as ps_a,
        ):
            xt = xt_pool.tile([128, KT * T], DT.float32r)
            for a in range(KT):
                nc.sync.dma_start(
                    out=xt[:, a * T:(a + 1) * T],
                    in_=xT.ap()[a * 128:(a + 1) * 128, :],
                )
            bq_sb = bias_pool.tile([128, H_PER_CORE], DT.float32)
            bk_sb = bias_pool.tile([128, H_PER_CORE], DT.float32)
            nc.sync.dma_start(out=bq_sb[:], in_=bq.ap())
            nc.sync.dma_start(out=bk_sb[:], in_=bk.ap())

            for w_dram, b_sb, spill in ((wq, bq_sb, qTs), (wk, bk_sb, kTs)):
                for h in range(H_PER_CORE):
                    wcol = wcol_pool.tile([128, KT, 128], DT.float32r, tag="wcol")
                    nc.sync.dma_start(
                        out=wcol[:],
                        in_=w_dram.ap()[:, h * 128:(h + 1) * 128].rearrange(
                            "(a p) c -> p a c", p=128
                        ),
                    )
                    for c in range(TQ):
                        ps = ps_a.tile([128, 512], DT.float32, tag="ps")
                        for a in range(KT):
                            nc.tensor.matmul(
                                ps[:],
                                wcol[:, a, :],
                                xt[:, a * T + c * 512: a * T + (c + 1) * 512],
                                start=(a == 0),
                                stop=(a == KT - 1),
                            )
                        stg = qk_stage.tile([128, 512], DT.float32r, tag="stg")
                        nc.scalar.activation(
                            stg[:], ps[:], AF.Identity, bias=b_sb[:, h:h + 1]
                        )
                        nc.sync.dma_start(
                            out=spill.ap()[h, :, c * 512:(c + 1) * 512], in_=stg[:]
                        )

        # ---------------- Phase A2: v = x @ wv (natural layout) ---------
        with (
            tc.tile_pool(name="xt2_pool", bufs=1) as xt2_pool,
            tc.tile_pool(name="wv_pool", bufs=1) as wv_pool,
            tc.tile_pool(name="v_stage", bufs=3) as v_stage,
            tc.tile_pool(name="ps_a2", bufs=4, space="PSUM") as ps_a2,
        ):
            xt2 = xt2_pool.tile([128, KT * T], DT.float32r)
            for a in range(KT):
                nc.sync.dma_start(
                    out=xt2[:, a * T:(a + 1) * T],
                    in_=xT.ap()[a * 128:(a + 1) * 128, :],
                )
            for half in range(2):
                wvh = wv_pool.tile([128, KT, 512], DT.float32r, tag="wvh")
                nc.sync.dma_start(
                    out=wvh[:],
                    in_=wv.ap()[:, half * 512:(half + 1) * 512].rearrange(
                        "(a p) c -> p a c", p=128
                    ),
                )
                for tt in range(TT):
                    ps = ps_a2.tile([128, 512], DT.float32, tag="psv")
                    for a in range(KT):
                        nc.tensor.matmul(
                            ps[:],
                            xt2[:, a * T + tt * 128: a * T + tt * 128 + 128],
                            wvh[:, a, :],
                            start=(a == 0),
                            stop=(a == KT - 1),
                        )
                    stg = v_stage.tile([128, 512], DT.float32r, tag="vstg")
                    nc.scalar.activation(stg[:], ps[:], AF.Copy)
                    nc.sync.dma_start(
                        out=vs.ap()[tt * 128:(tt + 1) * 128,
                                    half * 512:(half + 1) * 512],
                        in_=stg[:],
                    )

        # ---------------- Phase B: causal attention per head ------------
        with (
            tc.tile_pool(name="mask_pool", bufs=1) as mask_pool,
            tc.tile_pool(name="kv_pool", bufs=2) as kv_pool,
            tc.tile_pool(name="q_pool", bufs=2) as q_pool,
            tc.tile_pool(name="ex_pool", bufs=3) as ex_pool,
            tc.tile_pool(name="ot_pool", bufs=2) as ot_pool,
            tc.tile_pool(name="den_pool", bufs=2) as den_pool,
            tc.tile_pool(name="ps_s", bufs=2, space="PSUM") as ps_s,
            tc.tile_pool(name="ps_ot", bufs=2, space="PSUM") as ps_ot,
            tc.tile_pool(name="ps_den", bufs=2, space="PSUM") as ps_den,
            tc.tile_pool(name="ps_bc", bufs=2, space="PSUM") as ps_bc,
        ):
            mask_sb = mask_pool.tile([128, 1024], DT.float32)
            nc.sync.dma_start(out=mask_sb[:], in_=maskadd.ap())
            ones_sb = mask_pool.tile([128, 1], DT.float32r)
            nc.sync.dma_start(out=ones_sb[:], in_=ones_col.ap())
            onesr_sb = mask_pool.tile([1, 128], DT.float32r)
            nc.sync.dma_start(out=onesr_sb[:], in_=ones_row.ap())

            for h in range(H_PER_CORE):
                kt = kv_pool.tile([128, T], DT.float32r, tag="kt")
                nc.sync.dma_start(out=kt[:], in_=kTs.ap()[h])
                vt = kv_pool.tile([128, TT, 128], DT.float32r, tag="vt")
                nc.sync.dma_start(
                    out=vt[:],
                    in_=vs.ap()[:, h * 128:(h + 1) * 128].rearrange(
                        "(a p) c -> p a c", p=128
                    ),
                )
                for c in range(TQ):
                    qt = q_pool.tile([128, 512], DT.float32r, tag="qt")
                    nc.sync.dma_start(
                        out=qt[:], in_=qTs.ap()[h, :, c * 512:(c + 1) * 512]
                    )
                    otp = ps_ot.tile([128, 512], DT.float32, tag="otp")
                    den = ps_den.tile([1, 512], DT.float32, tag="den")
                    ntk = 4 * (c + 1)
                    for j in range(ntk):
                        sp = ps_s.tile([128, 512], DT.float32, tag="sp")
                        nc.tensor.matmul(
                            sp[:],
                            kt[:, j * 128:(j + 1) * 128],
                            qt[:],
                            start=True,
                            stop=True,
                        )
                        d = j * 128 - c * 512
                        if d >= 0:
                            nc.vector.tensor_add(
                                sp[:], sp[:], mask_sb[:, 512 - d:1024 - d]
                            )
                        ex = ex_pool.tile([128, 512], DT.float32r, tag="ex")
                        nc.scalar.activation(ex[:], sp[:], AF.Exp, scale=SCALE)
                        nc.tensor.matmul(
                            otp[:],
                            vt[:, j, :],
                            ex[:],
                            start=(j == 0),
                            stop=(j == ntk - 1),
                        )
                        nc.tensor.matmul(
                            den[:],
                            ones_sb[:],
                            ex[:],
                            start=(j == 0),
                            stop=(j == ntk - 1),
                        )
                    den_sb = den_pool.tile([1, 512], DT.float32, tag="den_sb")
                    nc.vector.tensor_copy(den_sb[:], den[:])
                    rec_sb = den_pool.tile([1, 512], DT.float32r, tag="rec_sb")
                    with nc.allow_low_precision(
                        reason="f32r rounding of softmax reciprocal is intended"
                    ):
                        nc.vector.reciprocal(rec_sb[:], den_sb[:])
                    bc = ps_bc.tile([128, 512], DT.float32, tag="bc")
                    nc.tensor.matmul(
                        bc[:], onesr_sb[:], rec_sb[:], start=True, stop=True
                    )
                    bc_sb = ot_pool.tile([128, 512], DT.float32, tag="bc_sb")
                    nc.scalar.activation(bc_sb[:], bc[:], AF.Copy)
                    ot_sb = ot_pool.tile([128, 512], DT.float32r, tag="ot_sb")
                    nc.vector.tensor_mul(ot_sb[:], otp[:], bc_sb[:])
                    nc.sync.dma_start(
                        out=oTs.ap()[h, :, c * 512:(c + 1) * 512], in_=ot_sb[:]
                    )

        # ---------------- Phase C: out = O @ wo --------------------------
        with (
            tc.tile_pool(name="wo_pool", bufs=1) as wo_pool,
            tc.tile_pool(name="oin_pool", bufs=2) as oin_pool,
            tc.tile_pool(name="co_stage", bufs=3) as co_stage,
            tc.tile_pool(name="ps_c", bufs=4, space="PSUM") as ps_c,
        ):
            wo_sb = wo_pool.tile([128, H_PER_CORE, T], DT.float32r)
            nc.sync.dma_start(
                out=wo_sb[:], in_=wo.ap().rearrange("(h p) d -> p h d", p=128)
            )
            for tt in range(TT):
                oin = oin_pool.tile([128, H_PER_CORE, 128], DT.float32r, tag="oin")
                nc.sync.dma_start(
                    out=oin[:],
                    in_=oTs.ap()[:, :, tt * 128:(tt + 1) * 128].rearrange(
                        "h p t -> p h t"
                    ),
                )
                for dc in range(4):
                    ps = ps_c.tile([128, 512], DT.float32, tag="psc")
                    for h in range(H_PER_CORE):
                        nc.tensor.matmul(
                            ps[:],
                            oin[:, h, :],
                            wo_sb[:, h, dc * 512:(dc + 1) * 512],
                            start=(h == 0),
                            stop=(h == H_PER_CORE - 1),
                        )
                    stg = co_stage.tile([128, 512], DT.float32, tag="cstg")
                    nc.scalar.activation(stg[:], ps[:], AF.Copy)
                    nc.sync.dma_start(
                        out=outp.ap()[tt * 128:(tt + 1) * 128,
                                      dc * 512:(dc + 1) * 512],
                        in_=stg[:],
                    )

    nc.compile()
    return nc


_NC_CACHE = {}


def _get_nc():
    if "nc" not in _NC_CACHE:
        _NC_CACHE["nc"] = build_nc()
    return _NC_CACHE["nc"]


def kernel(query, w_q, b_q, w_k, b_k, w_v, b_v, w_o, b_o, **kwargs):
    query = np.asarray(query, dtype=np.float32)
    w_q = np.asarray(w_q, dtype=np.float32)
    w_k = np.asarray(w_k, dtype=np.float32)
    w_v = np.asarray(w_v, dtype=np.float32)
    w_o = np.asarray(w_o, dtype=np.float32)
    b_q = np.asarray(b_q, dtype=np.float32)
    b_k = np.asarray(b_k, dtype=np.float32)
    b_v = np.asarray(b_v, dtype=np.float32)
    b_o = np.asarray(b_o, dtype=np.float32)

    # additive causal mask block: row p = key offset within a 128-tile,
    # col f (of 512-chunk) at diagonal offset dlt: valid iff f >= p + dlt.
    # slice [512-dlt : 1024-dlt] of this [128,1024] block gives the mask.
    g_idx = np.arange(1024)[None, :] - 512
    p_idx = np.arange(128)[:, None]
    maskadd = np.where(g_idx >= p_idx, 0.0, NEG).astype(np.float32)
    ones_col = np.ones((128, 1), dtype=np.float32)
    ones_row = np.ones((1, 128), dtype=np.float32)

    in_maps = []
    for core in range(8):
        b = core // 2
        g = core % 2
        s = slice(g * HD, (g + 1) * HD)
        in_maps.append(
            {
                "xT": np.ascontiguousarray(query[b].T),
                "wq": np.ascontiguousarray(w_q[:, s]),
                "wk": np.ascontiguousarray(w_k[:, s]),
                "wv": np.ascontiguousarray(w_v[:, s]),
                "wo": np.ascontiguousarray(w_o[s, :]),
                "bq": np.ascontiguousarray(
                    b_q[s].reshape(H_PER_CORE, 128).T
                ),
                "bk": np.ascontiguousarray(
                    b_k[s].reshape(H_PER_CORE, 128).T
                ),
                "maskadd": maskadd,
                "ones_col": ones_col,
                "ones_row": ones_row,
            }
        )

    global _LAST_IN_MAPS
    _LAST_IN_MAPS = in_maps
    nc = _get_nc()
    res = run_bass_kernel_spmd(nc, in_maps, core_ids=list(range(8)))

    out = np.zeros((B, T, D), dtype=np.float32)
    for core in range(8):
        out[core // 2] += res.results[core]["out"]
    out += (b_v @ w_o + b_o)[None, None, :]
    return out
